# revision 86
# baseline (speedup 1.0000x reference)
"""Trainium2 Bass kernel for nn_CrossAttention (B=8, L=K=512, M=N=P=D=64).

Math per batch b (one batch per NeuronCore, 8 cores):
  scoresT[k,l] = scale * (K @ Q^T)            # PE f32r, contract D=64
  ET = exp(scoresT)                           # ACT (softmax w/o max-sub: |s|<~45 safe in f32)
  vkc[k,n] = sum_p vk[k,p,n]*vexp[k,p]        # DVE mult(+bcast) + bf16 tree-reduce
  tmp[l,n|s] = ET_chunk^T @ [vkc | 1]         # PE, one PSUM accumulation group
                                              #   per l-chunk: lands directly in
                                              #   l-partition layout; col 64 is
                                              #   the softmax denominator
  attn[l,m] = sum_n vq[l,m,n]*tmp[l,n]        # DVE mult(+bcast) + bf16 tree-reduce
  x = attn/sums + q ; out = LN(x)*gamma+beta  # DVE/ACT

vk and vq are cast to bf16 on the host before upload (the value path is
bf16 on-device regardless, so numerics are unchanged) which halves their
HBM footprint and read traffic. All vq tiles are prefetched during the vk
phase (vq pool bufs=4).
"""

import numpy as np

B = 8
L = 512
KK = 512
MM = 64
NN = 64
PP = 64
DD = 64
NCORES = 8

_CACHE = {}


def _patch_multiwait_split():
    """This environment's walrus accepts only ONE sem-wait per instruction,
    while Tile emits instructions carrying several. Rewrite the BIR JSON just
    before compilation: hoist excess waits onto single-wait NoOps inserted
    immediately before the offending instruction on the same engine."""
    import json

    from concourse import bass_utils, bass2jax

    if getattr(bass_utils, "_multiwait_split_patched", False):
        return

    orig = bass_utils.compile_bir_kernel

    def _split(bir_json):
        if isinstance(bir_json, bytes):
            m = json.loads(bir_json.decode())
        else:
            m = json.loads(bir_json)
        cnt = 0
        for fn in m["functions"]:
            for bb in fn["blocks"]:
                insts = bb["instructions"]
                out = []
                for inst in insts:
                    si = inst.get("sync_info")
                    waits = si.get("on_wait", []) if si else []
                    if len(waits) > 1:
                        for w in waits[:-1]:
                            cnt += 1
                            out.append(
                                {
                                    "name": f"WS-{cnt}-{inst['name']}",
                                    "opcode": "NoOp",
                                    "engine": inst["engine"],
                                    "ins": [],
                                    "outs": [],
                                    "debug": inst.get("debug", 0),
                                    "sync_info": {
                                        "on_update": [],
                                        "on_wait": [w],
                                    },
                                }
                            )
                        si["on_wait"] = [waits[-1]]
                    out.append(inst)
                bb["instructions"] = out
        return json.dumps(m).encode()

    def patched(bir_json, tmpdir, neff_name="file.neff", **kw):
        return orig(_split(bir_json), tmpdir, neff_name=neff_name, **kw)

    bass_utils.compile_bir_kernel = patched
    bass2jax.compile_bir_kernel = patched
    bass_utils._multiwait_split_patched = True


def _build_nc(unroll=1):
    import contextlib

    import concourse.bass as bass
    import concourse.tile as tile
    from concourse import mybir
    from concourse.masks import make_identity

    _patch_multiwait_split()

    f32 = mybir.dt.float32
    f32r = mybir.dt.float32r
    bf16 = mybir.dt.bfloat16
    Alu = mybir.AluOpType
    Act = mybir.ActivationFunctionType

    nc = bass.Bass()
    q_d = nc.dram_tensor("q", [L, DD], f32, kind="ExternalInput")
    k_d = nc.dram_tensor("k", [KK, DD], f32, kind="ExternalInput")
    # vq/vk are cast to bf16 on the host (the value path is bf16 on-device
    # anyway — identical numerics to the previous in-flight SWDGE cast) so
    # their HBM footprint and read traffic halve.
    vq_d = nc.dram_tensor("vq", [L, MM * NN], bf16, kind="ExternalInput")
    vk_d = nc.dram_tensor("vk", [KK, PP * NN], bf16, kind="ExternalInput")
    # vexp is only read by the ACT broadcast copy whose output is bf16, so
    # a host-side bf16 cast is numerically identical and trims its DMA
    vexp_d = nc.dram_tensor("vexp", [KK, PP], bf16, kind="ExternalInput")
    scale_d = nc.dram_tensor("scale", [1, 1], f32, kind="ExternalInput")
    gamma_d = nc.dram_tensor("ln_gamma", [1, DD], f32, kind="ExternalInput")
    beta_d = nc.dram_tensor("ln_beta", [1, DD], f32, kind="ExternalInput")
    out_d = nc.dram_tensor("out", [L, MM], f32, kind="ExternalOutput")

    LT = L // 128   # 4 l-tiles
    KT = KK // 128  # 4 k-tiles
    HALF = PP // 2  # vk tiles stream as p-halves for earlier first compute

    with tile.TileContext(nc) as tc:
        lp_cm = nc.allow_low_precision("bf16 value path, f32r score/combine matmuls")
        with lp_cm:
            for it in range(unroll):
                _emit_body(
                    nc, tc, contextlib, mybir, make_identity,
                    f32, f32r, bf16, Alu, Act,
                    q_d, k_d, vq_d, vk_d, vexp_d, scale_d, gamma_d, beta_d,
                    out_d, LT, KT, HALF, it,
                )

    return nc


def _emit_body(
    nc, tc, contextlib, mybir, make_identity,
    f32, f32r, bf16, Alu, Act,
    q_d, k_d, vq_d, vk_d, vexp_d, scale_d, gamma_d, beta_d,
    out_d, LT, KT, HALF, it,
):
    with contextlib.ExitStack() as ctx:
        const = ctx.enter_context(tc.tile_pool(name=f"const{it}", bufs=1))
        vk_pool = ctx.enter_context(tc.tile_pool(name=f"vk{it}", bufs=4))
        vq_pool = ctx.enter_context(tc.tile_pool(name=f"vq{it}", bufs=4))
        prod_pool = ctx.enter_context(tc.tile_pool(name=f"prod{it}", bufs=3))
        small = ctx.enter_context(tc.tile_pool(name=f"small{it}", bufs=2))
        ps_scores = ctx.enter_context(
            tc.tile_pool(name=f"ps_s{it}", bufs=2, space="PSUM")
        )
        ps_acc = ctx.enter_context(tc.tile_pool(name=f"ps_acc{it}", bufs=1, space="PSUM"))
        ps_tr = ctx.enter_context(tc.tile_pool(name=f"ps_tr{it}", bufs=2, space="PSUM"))

        # ---- tiny inputs first (SP HWDGE queue): unblock step A asap ----
        # vexp lands in two pieces: k-tile 0's slice first (gates the very
        # first ACT broadcast -> first DVE multiply), then the rest.
        vexp_nat = const.tile([128, KT, PP], bf16)
        nc.sync.dma_start(
            out=vexp_nat[:, 0:1, :],
            in_=vexp_d[0:128, :].rearrange("(t p) d -> p t d", p=128),
        )
        nc.sync.dma_start(
            out=vexp_nat[:, 1:KT, :],
            in_=vexp_d[128:].rearrange("(t p) d -> p t d", p=128),
        )
        q_nat = const.tile([128, LT, DD], f32)
        nc.sync.dma_start(out=q_nat, in_=q_d[:].rearrange("(t p) d -> p t d", p=128))
        k_nat = const.tile([128, KT, DD], f32)
        nc.sync.dma_start(out=k_nat, in_=k_d[:].rearrange("(t p) d -> p t d", p=128))
        scale_bc = const.tile([128, 1], f32)
        nc.sync.dma_start(out=scale_bc, in_=scale_d[:].to_broadcast([128, 1]))
        gamma_bc = const.tile([128, DD], f32)
        nc.sync.dma_start(out=gamma_bc, in_=gamma_d[:].to_broadcast([128, DD]))
        beta_bc = const.tile([128, DD], f32)
        nc.sync.dma_start(out=beta_bc, in_=beta_d[:].to_broadcast([128, DD]))

        # ---- bulk inputs (Pool SWDGE queue — Pool is otherwise idle so
        # descriptor generation costs nothing; already bf16 in DRAM) ----
        # vk chunks first (step A consumes them in order), then all vq tiles
        # (consumed by step C after the tmpT barrier; bufs=4 keeps them all
        # resident so step C never waits on DMA).
        vk_chunks = []
        for i in range(KT):
            nchunk = 2
            csz = PP // nchunk
            chunks = []
            for cc in range(nchunk):
                vkh = vk_pool.tile([128, csz, NN], bf16, tag=f"vk{nchunk}")
                nc.gpsimd.dma_start(
                    out=vkh,
                    in_=vk_d[
                        i * 128 : (i + 1) * 128,
                        cc * csz * NN : (cc + 1) * csz * NN,
                    ],
                )
                chunks.append(vkh)
            vk_chunks.append(chunks)
        vq_tiles = []
        for j in range(LT):
            vq_t = vq_pool.tile([128, MM, NN], bf16, tag="vq")
            nc.gpsimd.dma_start(out=vq_t, in_=vq_d[j * 128 : (j + 1) * 128, :])
            vq_tiles.append(vq_t)

        # ---- constants ----
        identity = const.tile([128, 128], f32)
        # memset on DVE (idle at startup) so Pool's first work is the vk
        # SWDGE descriptor generation
        nc.vector.memset(identity, 0.0)
        make_identity(nc, identity, nomemset=True)
        zero_t = const.tile([128, 1], f32)
        nc.vector.memset(zero_t, 0.0)
        eps_t = const.tile([128, 1], f32)
        nc.vector.memset(eps_t, 1e-3)

        qT = const.tile([64, L], f32r)
        kT = const.tile([64, KK], f32r)
        ET = const.tile([128, KT, L], f32r)
        vkc = const.tile([128, KT, NN + 1], f32r)
        ones_t = const.tile([128, 1], f32)
        nc.vector.memset(ones_t, 1.0)
        # Memset can't write f32r; ACT Copy rounds f32 -> f32r legally.
        nc.scalar.activation(
            vkc[:, :, NN : NN + 1],
            ones_t[:, :, None].to_broadcast([128, KT, 1]),
            func=Act.Copy,
        )

        def emit_qkT():
            # qT, kT via PE transpose. Emitted after k-tile 0's broadcast
            # work so the ACT queue services the first bc (which gates the
            # first DVE multiply) before these copies.
            for i in range(LT):
                pq = ps_tr.tile([64, 128], f32, tag="tr")
                nc.tensor.transpose(pq, q_nat[:, i, :], identity)
                nc.scalar.copy(qT[:, i * 128 : (i + 1) * 128], pq)
            for i in range(KT):
                pk = ps_tr.tile([64, 128], f32, tag="tr")
                nc.tensor.transpose(pk, k_nat[:, i, :], identity)
                nc.scalar.copy(kT[:, i * 128 : (i + 1) * 128], pk)

        def emit_scores(i):
            # scoresT -> ET for k-tile i. Emitted late (after the NEXT
            # tile's broadcasts) — ET[i] is only consumed by the PE tmpT
            # accumulation, whose last term waits for vkc[3] anyway.
            ps_s = ps_scores.tile([128, L], f32, tag="sc")
            nc.tensor.matmul(
                ps_s,
                lhsT=kT[:, i * 128 : (i + 1) * 128],
                rhs=qT[:],
                start=True, stop=True,
            )
            nc.scalar.activation(
                ET[:, i, :], ps_s, func=Act.Exp, bias=zero_t[:], scale=scale_bc[:],
            )

        # ---- step A (vkc) ; vkc carries an extra ones-column (col 64) so
        # the tmpT matmul also produces the softmax denominators for free.
        for i in range(KT):
            # ACT materializes the vexp broadcast as packed bf16 so the DVE
            # multiply qualifies for 2x_1p (a stride-0 broadcast operand
            # would force 1 elem/cycle). ACT is otherwise idle during step A
            # and pipelines ahead of the DVE. Products for both halves land
            # in one tile so a single 6-level tree reduces p 64 -> 1.
            # k-tile 0 runs at quarter granularity so the first DVE multiply
            # starts as soon as the first quarter of vk has landed.
            nchunk = 2
            csz = PP // nchunk
            # the first tree level folds WITHIN each chunk so it can fire as
            # soon as that chunk's vk DMA lands, instead of waiting for the
            # whole k-tile (fills DVE idle during the DMA-paced ramp)
            fold = prod_pool.tile([128, PP // 2, NN], bf16, tag="fold")
            for cc in range(nchunk):
                bc = prod_pool.tile([128, csz, NN], bf16, tag=f"bc{nchunk}")
                nc.scalar.activation(
                    bc[:],
                    vexp_nat[
                        :, i, cc * csz : (cc + 1) * csz, None
                    ].to_broadcast([128, csz, NN]),
                    func=Act.Copy,
                )
                vkh = vk_chunks[i][cc]
                pr = prod_pool.tile([128, csz, NN], bf16, tag=f"prod{nchunk}")
                nc.vector.tensor_tensor(pr[:], vkh[:], bc[:], Alu.mult)
                h = csz // 2
                nc.vector.tensor_tensor(
                    fold[:, cc * h : (cc + 1) * h, :],
                    pr[:, 0:h, :], pr[:, h:csz, :], Alu.add,
                )
            # remaining tree levels over p: bf16 adds run at 2x
            cur = fold
            w = PP // 4
            while w >= 1:
                if w == 1:
                    nxt = vkc[:, i, 0:NN, None].rearrange("a b c -> a c b")
                else:
                    nxt = prod_pool.tile([128, w, NN], bf16, tag=f"atree{w}")
                nc.vector.tensor_tensor(
                    nxt[:], cur[:, 0:w, :], cur[:, w : 2 * w, :], Alu.add
                )
                cur = nxt
                w //= 2
            if i == 0:
                emit_qkT()
            emit_scores(i)

        # ---- tmp[l, n|sum] = ET_chunk^T @ [vkc | 1], one PSUM accumulation
        # group per l-chunk: the result lands DIRECTLY in l-partition layout
        # (no tmpT transposes, no sums-row transposes, no PSUM->SBUF spill
        # of the [65, 512] intermediate). Column 64 is the softmax
        # denominator for those l rows. Each tile is bank-padded so the four
        # concurrent accumulation groups sit in distinct PSUM zero regions.
        ps_tmp = []
        for j in range(LT):
            ps_tmp_j = ps_acc.tile([128, 512], f32, tag=f"tmpj{j}")
            ps_tmp.append(ps_tmp_j)
        for i in range(KT):
            for j in range(LT):
                # plain f32 here: fp32r has ISA restrictions at this
                # moving size (s3d3_mm_fp32r_restrictions), and 65 cols x 4
                # cyc/row is equally cheap
                nc.tensor.matmul(
                    ps_tmp[j][:, 0 : NN + 1],
                    lhsT=ET[:, i, j * 128 : (j + 1) * 128].bitcast(f32),
                    rhs=vkc[:, i, :].bitcast(f32),
                    start=(i == 0), stop=(i == KT - 1),
                )
        tmp_all = const.tile([128, LT, NN], bf16)
        recip_col = const.tile([128, LT], f32)
        for j in range(LT):
            # j=0 gates step C's first multiply: copy it on the DVE, which
            # is idle at this barrier (ACT may still be draining step-A
            # broadcasts); the rest go to ACT
            if j == 0:
                nc.vector.tensor_copy(tmp_all[:, j, :], ps_tmp[j][:, 0:NN])
            else:
                nc.scalar.copy(tmp_all[:, j, :], ps_tmp[j][:, 0:NN])
            nc.vector.reciprocal(
                recip_col[:, j : j + 1], ps_tmp[j][:, NN : NN + 1]
            )

        # ---- per l-tile: step C, rescale+residual, LayerNorm ----
        for j in range(LT):
            vq_t = vq_tiles[j]
            pr2 = prod_pool.tile([128, MM, NN], bf16, tag="prod2")
            nc.vector.tensor_tensor(
                pr2[:],
                vq_t[:],
                tmp_all[:, j, None, :].to_broadcast([128, MM, NN]),
                Alu.mult,
            )
            attn = small.tile([128, MM], f32, tag="attn")
            # tree-reduce over n (inner axis): bf16 adds at 2x
            cur = pr2
            w = NN // 2
            while w >= 1:
                if w == 1:
                    nxt = attn[:, :, None]
                else:
                    nxt = prod_pool.tile([128, MM, w], bf16, tag=f"ctree{w}")
                nc.vector.tensor_tensor(
                    nxt[:], cur[:, :, 0:w], cur[:, :, w : 2 * w], Alu.add
                )
                cur = nxt
                w //= 2

            # x = attn * (1/sums) + q
            x = small.tile([128, MM], f32, tag="x")
            nc.vector.scalar_tensor_tensor(
                out=x, in0=attn, scalar=recip_col[:, j : j + 1],
                in1=q_nat[:, j, :], op0=Alu.mult, op1=Alu.add,
            )

            # LayerNorm(eps=1e-3)
            stats = small.tile([128, 6], f32, tag="stats")
            nc.vector.bn_stats(out=stats, in_=x[:])
            mv = small.tile([128, 2], f32, tag="mv")
            nc.vector.bn_aggr(out=mv, in_=stats[:])
            sd = small.tile([128, 1], f32, tag="sd")
            nc.scalar.activation(
                sd, mv[:, 1:2], func=Act.Sqrt, bias=eps_t[:], scale=1.0
            )
            rstd = small.tile([128, 1], f32, tag="rstd")
            nc.vector.reciprocal(rstd, sd)
            xn = small.tile([128, MM], f32, tag="xn")
            nc.vector.tensor_scalar(
                out=xn, in0=x, scalar1=mv[:, 0:1], scalar2=rstd,
                op0=Alu.subtract, op1=Alu.mult,
            )
            # gamma/beta applications run on Pool (idle this late), keeping
            # the DVE on the value-path trees; the final tile stays on the
            # DVE to avoid cross-engine hops on the terminal store chain
            eng = nc.vector if j == LT - 1 else nc.gpsimd
            xg = small.tile([128, MM], f32, tag="xg")
            eng.tensor_tensor(xg, xn, gamma_bc, Alu.mult)
            out_t = small.tile([128, MM], f32, tag="out_t")
            eng.tensor_tensor(out_t, xg, beta_bc, Alu.add)

            # earlier tiles store via Pool SWDGE (generation on the idle
            # Pool engine); the final store rides SP HWDGE with no earlier
            # generation queued ahead of it on the shared HWDGE device
            seng = nc.sync if j == LT - 1 else nc.gpsimd
            seng.dma_start(out=out_d[j * 128 : (j + 1) * 128, :], in_=out_t)


def _get_nc():
    if "nc" not in _CACHE:
        _CACHE["nc"] = _build_nc()
    return _CACHE["nc"]


def kernel(q, k, vq, vk, vexp, scale, ln_gamma, ln_beta):
    from concourse import bass_utils

    nc = _get_nc()
    import ml_dtypes

    q = np.ascontiguousarray(np.asarray(q, dtype=np.float32))
    k = np.ascontiguousarray(np.asarray(k, dtype=np.float32))
    vq = np.ascontiguousarray(
        np.asarray(vq, dtype=np.float32).astype(ml_dtypes.bfloat16)
    ).reshape(B, L, MM * NN)
    vk = np.ascontiguousarray(
        np.asarray(vk, dtype=np.float32).astype(ml_dtypes.bfloat16)
    ).reshape(B, KK, PP * NN)
    vexp = np.ascontiguousarray(
        np.asarray(vexp, dtype=np.float32).astype(ml_dtypes.bfloat16)
    )
    scale_arr = np.asarray(scale, dtype=np.float32).reshape(1, 1)
    gamma_arr = np.asarray(ln_gamma, dtype=np.float32).reshape(1, DD)
    beta_arr = np.asarray(ln_beta, dtype=np.float32).reshape(1, DD)

    in_maps = [
        {
            "q": q[c],
            "k": k[c],
            "vq": vq[c],
            "vk": vk[c],
            "vexp": vexp[c],
            "scale": scale_arr,
            "ln_gamma": gamma_arr,
            "ln_beta": beta_arr,
        }
        for c in range(NCORES)
    ]
    res = bass_utils.run_bass_kernel_spmd(nc, in_maps, core_ids=list(range(NCORES)))
    out = np.stack([res.results[c]["out"] for c in range(NCORES)], axis=0)
    return out.astype(np.float32)
